# revision 67
# baseline (speedup 1.0000x reference)
"""Causal linear attention (elu+1 feature map) Trainium2 Bass kernel.

Full inputs q,k,v: [4, 2048, 12, 64] fp32 -> out [4, 2048, 12, 64] fp32.
Sharding: 48 (batch, head) pairs, 6 per core across 8 NeuronCores; each
core processes 3 "pair-pairs" (2 pairs stacked on the 128 partitions).

Math per (n,h) pair, per 128-chunk g (exact algebraic regrouping):
  phi(x) = elu(x)+1 = max(x+1, min(exp(x), 1))   [exp first: no pre-min]
  S_g    = sum_{g'<=g} kf_chunk^T v_chunk        (PSUM kv + segmented scan)
  K1_g   = sum_{g'<=g} kf_chunk^T 1
  scT    = kfT^T qfT, masked to s<=c
  num    = qf @ S_{g-1} + scT^T @ v              (PSUM accumulate)
  z      = qf @ K1_{g-1} + scT^T @ 1
  out    = num / z

All matmuls in bf16. Engine balance (per pair-pair):
  ACT:  exp(k), exp(q), score evacuations, one kn evac
  DVE:  phi min/add, merged kv scan, k1c scan, recip, out normalize,
        one kn evac, bf16 mask-mults
  Pool: phi max-combines, affine_select masks
  PE:   all matmuls + per-chunk transposes of phi(k)
Emission is software-pipelined: next-pp prologue ops are interleaved
into the current pp's phase stream so per-engine FIFOs don't stall.
"""

import json
import os

import numpy as np

# ---------------------------------------------------------------------------
# Workaround for walrus "Too many sync wait commands": cap waits per
# instruction at 1, hoisting overflow onto same-engine NoOps inserted
# immediately before (engines run their stream in order, so consecutive
# waits AND together identically).
# ---------------------------------------------------------------------------
_wsplit_counter = [0]


def _split_instruction_waits(inst):
    si = inst.get("sync_info")
    if not si:
        return []
    waits = si.get("on_wait") or []
    if len(waits) <= 1:
        return []
    si["on_wait"] = waits[-1:]
    nops = []
    for w in waits[:-1]:
        _wsplit_counter[0] += 1
        nops.append(
            {
                "debug": inst.get("debug", 0),
                "engine": inst["engine"],
                "ins": [],
                "name": f"I-wsplit-{_wsplit_counter[0]}",
                "opcode": "NoOp",
                "outs": [],
                "sync_info": {"on_update": [], "on_wait": [w]},
            }
        )
    return nops


def _fix_module_json(raw: bytes) -> bytes:
    m = json.loads(raw)
    changed = False
    for f in m.get("functions", []):
        for b in f.get("blocks", []):
            out = []
            for inst in b.get("instructions", []):
                nops = _split_instruction_waits(inst)
                if nops:
                    changed = True
                    out.extend(nops)
                out.append(inst)
            b["instructions"] = out
    return json.dumps(m).encode() if changed else raw


_patch_installed = [False]


def _install_bir_patch():
    if _patch_installed[0]:
        return
    _patch_installed[0] = True
    import concourse.bass as _bass

    _orig = _bass.Bass.to_json_bytes

    def _patched(self):
        return _fix_module_json(_orig(self))

    _bass.Bass.to_json_bytes = _patched


# ---------------------------------------------------------------------------
# Problem constants (hardcoded per contest contract)
# ---------------------------------------------------------------------------
B, L, H, D = 4, 2048, 12, 64
CHUNK = 128
G = L // CHUNK  # 16
N_CORES = 8
PAIRS = [(n, h) for n in range(B) for h in range(H)]  # 48
PER_CORE = len(PAIRS) // N_CORES  # 6
NPP = PER_CORE // 2  # 3 pair-pairs per core

# ---------------------------------------------------------------------------
# Engine-assignment knobs (tuned against TimelineSim)
# ---------------------------------------------------------------------------
# kn evacuation engine per (s, half): "act" | "dve"
KN_ENG = {(pp, s, half): "dve" for pp in range(NPP) for s in (0, 1)
          for half in (0, 1)}
for _pp in range(NPP):
    KN_ENG[(_pp, 0, 0)] = "act"
KN_ENG[(2, 1, 0)] = "act"
KN_ENG[(2, 1, 1)] = "act"
KN_ENG[(2, 0, 1)] = "act"
KN_ENG[(1, 1, 1)] = "act"
# score mask mode per (pp, s, h): "b" = ACT evac + DVE bf16 mul,
#   "a" = ACT evac + Pool affine_select, "d" = DVE direct fp32 mul
MASK_MODE = {(pp, s, h): "a" for pp in range(NPP) for s in (0, 1)
             for h in (0, 1)}
for _s in (0, 1):
    for _h in (0, 1):
        MASK_MODE[(NPP - 1, _s, _h)] = "b"
PHASE_ORDER = [(0, 0), (1, 0), (0, 1), (1, 1)]
INS_BUFS = 2
WORK_BUFS = 6
OUTSB_BUFS = 2


def _build_bass():
    import concourse.bass as bass
    import concourse.tile as tile
    import concourse.mybir as mybir

    fp32 = mybir.dt.float32
    bf16 = mybir.dt.bfloat16
    AF = mybir.ActivationFunctionType
    ALU = mybir.AluOpType

    nc = bass.Bass()
    qt = nc.dram_tensor("qt", [NPP, 128, L], bf16, kind="ExternalInput")
    kt = nc.dram_tensor("kt", [NPP, 128, L], bf16, kind="ExternalInput")
    vt = nc.dram_tensor("vt", [NPP, 128, 2, G, D], bf16, kind="ExternalInput")
    mask = nc.dram_tensor("mask", [128, CHUNK], bf16, kind="ExternalInput")
    ident = nc.dram_tensor("ident", [128, D], bf16, kind="ExternalInput")
    on = nc.dram_tensor("on", [NPP, 128, 2, G, D], bf16, kind="ExternalOutput")

    with tile.TileContext(nc) as tc:
        with (
            tc.tile_pool(name="singles", bufs=1) as singles,
            tc.tile_pool(name="ins", bufs=INS_BUFS) as ins,
            tc.tile_pool(name="work", bufs=WORK_BUFS) as work,
            tc.tile_pool(name="ps_knt", bufs=1, space="PSUM") as ps_knt,
            tc.tile_pool(name="ps_kv", bufs=1, space="PSUM") as ps_kv,
            tc.tile_pool(name="ps_kz", bufs=1, space="PSUM") as ps_kz,
            tc.tile_pool(name="ps_sc", bufs=1, space="PSUM") as ps_sc,
            tc.tile_pool(name="ps_num", bufs=2, space="PSUM") as ps_num,
        ):
            st = {}

            def proA(pp, split_k=False):
                """Issue input DMAs for pair-pair pp (k first: exp(k) is the
                head of the longest dependency chain). k and q share one
                [128, 2, L] tile."""
                d = st.setdefault(pp, {})
                kq = ins.tile([128, 2, L], bf16, tag="kq", name=f"kq{pp}")
                vv = ins.tile([128, 2, G, D], bf16, tag="vv", name=f"vv{pp}")
                if split_k:
                    # halves let exp(k) start after half the transfer
                    nc.sync.dma_start(out=kq[:, 0, 0 : L // 2],
                                      in_=kt[pp, :, 0 : L // 2])
                    nc.sync.dma_start(out=kq[:, 0, L // 2 : L],
                                      in_=kt[pp, :, L // 2 : L])
                else:
                    nc.sync.dma_start(out=kq[:, 0], in_=kt[pp])
                # q on the sync queue too: a DGE setup on the ACT queue
                # would delay exp dispatch by ~0.7us
                nc.sync.dma_start(out=kq[:, 1], in_=qt[pp])
                nc.sync.dma_start(out=vv[:], in_=vt[pp])
                d["kq"], d["vv"] = kq, vv

            def singles_init():
                d = st.setdefault("S", {})
                identT = singles.tile([128, D], bf16)
                nc.sync.dma_start(out=identT[:], in_=ident[:])
                maskb = singles.tile([128, CHUNK], bf16)
                nc.sync.dma_start(out=maskb[:], in_=mask[:])
                onesb = singles.tile([128, 1], bf16)
                nc.vector.memset(onesb[:], 1.0)
                negone = singles.tile([128, 1], fp32)
                nc.gpsimd.memset(negone[:], -1.0)
                d["negone"] = negone
                # merged-scan segment mask over free dims (h=2, m=64, g=8):
                # 0 at (h, m, g=0) resets the running state per (h, m) row,
                # giving independent per-h chunk prefixes in one scan.
                seg = singles.tile([128, 2, D, 8], bf16)
                nc.vector.memset(seg[:], 1.0)
                nc.vector.memset(seg[:, :, :, 0:1], 0.0)
                seg16 = singles.tile([128, G], bf16)
                nc.vector.memset(seg16[:], 1.0)
                nc.vector.memset(seg16[:, 0:1], 0.0)
                d["maskb"], d["identT"], d["onesb"] = maskb, identT, onesb
                d["seg"], d["seg16"] = seg, seg16

            def phi_exp(pp, t, half=None):
                """ACT: e = exp(x) straight from the DMA'd input."""
                d = st[pp]
                ti = 0 if t == "k" else 1
                if "e" not in d:
                    d["e"] = work.tile([128, 2, L], bf16, tag="e",
                                       name=f"e{pp}")
                cs = slice(None) if half is None else \
                    slice(half[0] * (L // half[1]),
                          (half[0] + 1) * (L // half[1]))
                nc.scalar.activation(out=d["e"][:, ti, cs],
                                     in_=d["kq"][:, ti, cs], func=AF.Exp,
                                     bias=st["S"]["negone"][:])

            def phi_mam(pp, t, half=None):
                """DVE in place: e = min(e,1); x = x+1; x = max(x, e) = phi.
                After this kq[:, 0] = kf, kq[:, 1] = qf."""
                d = st[pp]
                ti = 0 if t == "k" else 1
                cs = slice(None) if half is None else \
                    slice(half[0] * (L // half[1]),
                          (half[0] + 1) * (L // half[1]))
                e, kq = d["e"], d["kq"]
                nc.vector.tensor_scalar_min(out=e[:, ti, cs],
                                            in0=e[:, ti, cs], scalar1=1.0)
                nc.vector.tensor_max(out=kq[:, ti, cs], in0=kq[:, ti, cs],
                                     in1=e[:, ti, cs])

            def proPE_T(pp, s, half, hb):
                """Transposes of phi(k) chunks for one h-half of partition
                half s. One knp tile [128, 2(hb), 8, D] per pp: the two hb
                slots ping-pong within a single PSUM bank (region-level
                dependency tracking), so transpose/evac/kv overlap."""
                d = st[pp]
                S = st["S"]
                kq = d["kq"]
                if "knp" not in d:
                    d["knp"] = ps_knt.tile([128, 2, 8, D], bf16, tag="knp",
                                           name=f"knp{pp}")
                knp = d["knp"]
                d[f"knph{s}_{half}"] = hb
                po = D * s
                for i, g in enumerate(range(half * 8, half * 8 + 8)):
                    nc.tensor.transpose(
                        knp[:, hb, i, :],
                        kq[po : po + D, 0, g * CHUNK : (g + 1) * CHUNK],
                        S["identT"][po : po + D, :],
                    )

            def kn_evac(pp, s, half):
                d = st[pp]
                knp = d["knp"]
                hb = d[f"knph{s}_{half}"]
                key = f"kn{s}"
                if key not in d:
                    d[key] = ins.tile([128, G, D], bf16, tag=f"kn{s}",
                                      name=f"kn{pp}_{s}")
                kn = d[key]
                gs = slice(half * 8, half * 8 + 8)
                if KN_ENG[(pp, s, half)] == "act":
                    nc.scalar.copy(out=kn[:, gs], in_=knp[:, hb])
                else:
                    nc.vector.tensor_copy(out=kn[:, gs], in_=knp[:, hb])

            def proPE_KV(pp, s, half):
                """kv products for one h-half of partition half s into
                kvp[h, m, g] slots + chunk k1 sums into kzp[:, 0:G]."""
                d = st[pp]
                S = st["S"]
                vv, kn = d["vv"], d[f"kn{s}"]
                if "kvp" not in d:
                    d["kvp"] = ps_kv.tile([128, 2, D, 8], fp32, tag="kv",
                                          name=f"kv{pp}")
                    d["kzp"] = ps_kz.tile([128, 48], fp32, tag="kz",
                                          name=f"kz{pp}")
                kvp, kzp = d["kvp"], d["kzp"]
                po = D * s
                for g in range(half * 8, half * 8 + 8):
                    h, gh = divmod(g, 8)
                    nc.tensor.matmul(
                        kvp[po : po + D, h, :, gh], kn[:, g, :],
                        vv[:, s, g, :],
                        start=(gh == 0), stop=(gh == 7),
                        skip_group_check=True)
                    nc.tensor.matmul(
                        kzp[po : po + D, g : g + 1], kn[:, g, :],
                        S["onesb"][:],
                        start=(g == 0), stop=False, skip_group_check=True)

            def scans(pp):
                """DVE: one segmented prefix scan over (h, m, g) — the seg
                reset at g=0 per (h, m) gives independent per-h chunk
                prefixes — plus the k1c scan. The h=0 total is NOT folded
                into h=1 here; phases add it via an extra PE matmul."""
                d = st[pp]
                S = st["S"]
                kvp, kzp = d["kvp"], d["kzp"]
                kvs = ins.tile([128, 2, D, 8], bf16, tag="kvs",
                               name=f"kvs{pp}")
                k1c = work.tile([128, G], bf16, tag="k1c", name=f"k1c{pp}")
                # k1c first: it unblocks the z-inter matmuls and is tiny
                nc.vector.tensor_tensor_scan(
                    out=k1c[:], data0=S["seg16"][:], data1=kzp[:, 0:G],
                    initial=0.0, op0=ALU.mult, op1=ALU.add)
                nc.vector.tensor_tensor_scan(
                    out=kvs[:].rearrange("p h m g -> p (h m g)"),
                    data0=S["seg"][:].rearrange("p h m g -> p (h m g)"),
                    data1=kvp[:].rearrange("p h m g -> p (h m g)"),
                    initial=0.0, op0=ALU.mult, op1=ALU.add)
                d["kvs"], d["k1c"] = kvs, k1c

            def phase_block(pp, pi):
                """One phase: 8 chunks of half s, h-group h."""
                s, h = PHASE_ORDER[pi]
                d = st[pp]
                S = st["S"]
                kq = d["kq"]
                vv, kvs, k1c, kzp = d["vv"], d["kvs"], d["k1c"], d["kzp"]
                if "outsb" not in d:
                    d["outsb"] = ins.tile([128, 2, G, D], bf16, tag="outsb",
                                          name=f"outsb{pp}", bufs=OUTSB_BUFS)
                outsb = d["outsb"]
                po = D * s
                zc = 16 + 16 * s
                hg = slice(8 * h, 8 * h + 8)
                nums = ps_num.tile([128, 8, D], fp32, tag="num",
                                   name=f"num{pp}_{s}_{h}")
                mmode = MASK_MODE[(pp, s, h)]
                for sq in (0, 1):
                    # 4-chunk sub-phases: 1-bank scp tiles double-buffer in
                    # the 2-bank pool, breaking the evac->sc serialization
                    scp = ps_sc.tile([128, 4, CHUNK], fp32, tag="sc",
                                     name=f"sc{pp}_{s}_{h}_{sq}")
                    i0 = 4 * sq
                    for i in range(i0, i0 + 4):
                        g = 8 * h + i
                        cs = slice(g * CHUNK, (g + 1) * CHUNK)
                        nc.tensor.matmul(
                            scp[:, i - i0, :], kq[po : po + D, 0, cs],
                            kq[po : po + D, 1, cs],
                            start=True, stop=True, skip_group_check=True)
                    for i in range(i0, i0 + 4):
                        g = 8 * h + i
                        if g == 0:
                            continue
                        cs = slice(g * CHUNK, (g + 1) * CHUNK)
                        hp, ghp = divmod(g - 1, 8)
                        nc.tensor.matmul(
                            nums[:, i, :], kq[po : po + D, 1, cs],
                            kvs[po : po + D, hp, :, ghp],
                            start=(i == 0 or g == 1), stop=False,
                            skip_group_check=True)
                        if hp == 1:
                            # per-h prefixes: add h=0 total for h=1 chunks
                            nc.tensor.matmul(
                                nums[:, i, :], kq[po : po + D, 1, cs],
                                kvs[po : po + D, 0, :, 7 : 8].rearrange(
                                    "p m g -> p (m g)"),
                                start=False, stop=False,
                                skip_group_check=True)
                        nc.tensor.matmul(
                            kzp[:, zc + g : zc + g + 1],
                            kq[po : po + D, 1, cs],
                            k1c[po : po + D, g - 1 : g],
                            start=False, stop=False,
                            skip_group_check=True)
                    scb = work.tile([128, 4, CHUNK], bf16, tag="scb",
                                    name=f"scb{pp}_{s}_{h}_{sq}")
                    if mmode == "d":
                        mb = S["maskb"][:].unsqueeze(1).broadcast_to(
                            [128, 4, CHUNK])
                        nc.vector.tensor_mul(out=scb[:], in0=scp[:], in1=mb)
                    else:
                        scf = work.tile([128, 4, CHUNK], bf16, tag="scf",
                                        name=f"scf{pp}_{s}_{h}_{sq}")
                        nc.scalar.copy(out=scf[:], in_=scp[:])
                        if mmode == "a":
                            nc.gpsimd.affine_select(
                                out=scb[:], in_=scf[:],
                                pattern=[[0, 4], [1, CHUNK]],
                                compare_op=ALU.is_ge, fill=0.0, base=0,
                                channel_multiplier=-1)
                        else:
                            mb = S["maskb"][:].unsqueeze(1).broadcast_to(
                                [128, 4, CHUNK])
                            nc.vector.tensor_mul(out=scb[:], in0=scf[:],
                                                 in1=mb)
                    for i in range(i0, i0 + 4):
                        g = 8 * h + i
                        nc.tensor.matmul(
                            kzp[:, zc + g : zc + g + 1], scb[:, i - i0, :],
                            S["onesb"][:],
                            start=False,
                            stop=(s == 1 and g == G - 1),
                            skip_group_check=True)
                    for i in range(i0, i0 + 4):
                        g = 8 * h + i
                        nc.tensor.matmul(
                            nums[:, i, :], scb[:, i - i0, :],
                            vv[:, s, g, :],
                            start=False, stop=(i == 7),
                            skip_group_check=True)
                r4 = work.tile([128, 8], fp32, tag="r4",
                               name=f"r4{pp}_{s}_{h}")
                nc.vector.reciprocal(
                    out=r4[:],
                    in_=kzp[:, zc + 8 * h : zc + 8 * h + 8])
                rb = r4[:].unsqueeze(2).broadcast_to([128, 8, D])
                nc.vector.tensor_mul(out=outsb[:, s, hg],
                                     in0=nums[:], in1=rb)
                nc.sync.dma_start(out=on[pp, :, s, hg],
                                  in_=outsb[:, s, hg])

            # ---------------- emission schedule ----------------
            # Startup: pp0's k-path races ahead at half-tile granularity so
            # the transpose/kv/scan chain starts as early as possible, with
            # exp(q)/phi(q) filling ACT/DVE gaps.
            def pro_tail(pp):
                """Transpose/evac/kv chain. The s=1 transposes depend only
                on phi columns (not the s=0 chain), so the rounds go
                (s0,h0),(s1,h0),(s0,h1),(s1,h1) with alternating ping-pong
                slots."""
                proPE_T(pp, 0, 0, 0)
                kn_evac(pp, 0, 0)
                proPE_T(pp, 1, 0, 1)
                kn_evac(pp, 1, 0)
                proPE_KV(pp, 0, 0)
                proPE_T(pp, 0, 1, 0)
                proPE_KV(pp, 1, 0)
                kn_evac(pp, 0, 1)
                proPE_T(pp, 1, 1, 1)
                kn_evac(pp, 1, 1)
                proPE_KV(pp, 0, 1)
                proPE_KV(pp, 1, 1)
                scans(pp)

            proA(0, split_k=True)
            singles_init()
            phi_exp(0, "k", (0, 2))
            phi_mam(0, "k", (0, 2))
            phi_exp(0, "k", (1, 2))
            proPE_T(0, 0, 0, 0)
            kn_evac(0, 0, 0)
            phi_mam(0, "k", (1, 2))
            proPE_T(0, 1, 0, 1)
            kn_evac(0, 1, 0)
            proPE_KV(0, 0, 0)
            phi_exp(0, "q")
            proPE_T(0, 0, 1, 0)
            proPE_KV(0, 1, 0)
            kn_evac(0, 0, 1)
            phi_mam(0, "q")
            proPE_T(0, 1, 1, 1)
            kn_evac(0, 1, 1)
            proPE_KV(0, 0, 1)
            proPE_KV(0, 1, 1)
            scans(0)

            for pp in range(NPP):
                nxt = pp + 1 if pp + 1 < NPP else None
                if nxt is not None:
                    proA(nxt)
                phase_block(pp, 0)
                phase_block(pp, 1)
                if nxt is not None:
                    phi_exp(nxt, "k")
                    phi_mam(nxt, "k")
                phase_block(pp, 2)
                if nxt is not None:
                    phi_exp(nxt, "q")
                phase_block(pp, 3)
                if nxt is not None:
                    phi_mam(nxt, "q")
                    pro_tail(nxt)

    return nc


_cached = {}


def _prep_inputs(q, k, v):
    import ml_dtypes

    bf = ml_dtypes.bfloat16
    # [s, c] : 1 if s<=c (bf16 exact 0/1)
    maskarr = np.ascontiguousarray(
        np.tril(np.ones((CHUNK, CHUNK), np.float32)).T
    ).astype(bf)
    identarr = np.concatenate([np.eye(D, dtype=np.float32)] * 2, axis=0).astype(bf)
    in_maps = []
    for c in range(N_CORES):
        sel = PAIRS[c * PER_CORE : (c + 1) * PER_CORE]
        qtl = np.empty((NPP, 128, L), bf)
        ktl = np.empty((NPP, 128, L), bf)
        vtl = np.empty((NPP, 128, 2, G, D), bf)
        for j in range(NPP):
            for s in (0, 1):
                n, h = sel[2 * j + s]
                # ship x+1: saves the +1 DVE pass on device (exp uses
                # the ACT bias to undo the shift)
                qtl[j, D * s : D * s + D] = (q[n, :, h, :].T + 1.0).astype(bf)
                ktl[j, D * s : D * s + D] = (k[n, :, h, :].T + 1.0).astype(bf)
                vtl[j, :, s] = (
                    v[n, :, h, :].reshape(G, CHUNK, D).transpose(1, 0, 2)
                ).astype(bf)
        in_maps.append(
            {"qt": qtl, "kt": ktl, "vt": vtl, "mask": maskarr, "ident": identarr}
        )
    return in_maps


def kernel(q: np.ndarray, k: np.ndarray, v: np.ndarray) -> np.ndarray:
    _install_bir_patch()
    from concourse.bass_utils import run_bass_kernel_spmd

    if "nc" not in _cached:
        _cached["nc"] = _build_bass()
    nc = _cached["nc"]

    in_maps = _prep_inputs(q, k, v)
    try:
        res = run_bass_kernel_spmd(nc, in_maps, core_ids=list(range(N_CORES)))
    except ModuleNotFoundError:
        # BASS_TRACE=1 with no axon NTFF hook in the container: retry untraced
        os.environ["BASS_NEVER_TRACE"] = "1"
        res = run_bass_kernel_spmd(nc, in_maps, core_ids=list(range(N_CORES)))
    _cached["last_result"] = res

    out = np.empty((B, L, H, D), np.float32)
    for c in range(N_CORES):
        sel = PAIRS[c * PER_CORE : (c + 1) * PER_CORE]
        o = res.results[c]["on"]  # [NPP, 128, 2, G, D] bf16
        for j in range(NPP):
            for s in (0, 1):
                n, h = sel[2 * j + s]
                # [c, G, D] -> [L, D]
                out[n, :, h, :] = (
                    o[j, :, s].astype(np.float32).transpose(1, 0, 2).reshape(L, D)
                )
    return out


# revision 68
# speedup vs baseline: 1.1761x; 1.1761x over previous
"""Causal linear attention (elu+1 feature map) Trainium2 Bass kernel.

Full inputs q,k,v: [4, 2048, 12, 64] fp32 -> out [4, 2048, 12, 64] fp32.
Sharding: 48 (batch, head) pairs, 6 per core across 8 NeuronCores; each
core processes 3 "pair-pairs" (2 pairs stacked on the 128 partitions).

Math per (n,h) pair, per 128-chunk g (exact algebraic regrouping):
  phi(x) = elu(x)+1 = max(x+1, min(exp(x), 1))   [exp first: no pre-min]
  S_g    = sum_{g'<=g} kf_chunk^T v_chunk        (PSUM kv + segmented scan)
  K1_g   = sum_{g'<=g} kf_chunk^T 1
  scT    = kfT^T qfT, masked to s<=c
  num    = qf @ S_{g-1} + scT^T @ v              (PSUM accumulate)
  z      = qf @ K1_{g-1} + scT^T @ 1
  out    = num / z

All matmuls in bf16. Engine balance (per pair-pair):
  ACT:  exp(k), exp(q), score evacuations, one kn evac
  DVE:  phi min/add, merged kv scan, k1c scan, recip, out normalize,
        one kn evac, bf16 mask-mults
  Pool: phi max-combines, affine_select masks
  PE:   all matmuls + per-chunk transposes of phi(k)
Emission is software-pipelined: next-pp prologue ops are interleaved
into the current pp's phase stream so per-engine FIFOs don't stall.
"""

import json
import os

import numpy as np

# ---------------------------------------------------------------------------
# Workaround for walrus "Too many sync wait commands": cap waits per
# instruction at 1, hoisting overflow onto same-engine NoOps inserted
# immediately before (engines run their stream in order, so consecutive
# waits AND together identically).
# ---------------------------------------------------------------------------
_wsplit_counter = [0]


def _split_instruction_waits(inst):
    si = inst.get("sync_info")
    if not si:
        return []
    waits = si.get("on_wait") or []
    if len(waits) <= 1:
        return []
    si["on_wait"] = waits[-1:]
    nops = []
    for w in waits[:-1]:
        _wsplit_counter[0] += 1
        nops.append(
            {
                "debug": inst.get("debug", 0),
                "engine": inst["engine"],
                "ins": [],
                "name": f"I-wsplit-{_wsplit_counter[0]}",
                "opcode": "NoOp",
                "outs": [],
                "sync_info": {"on_update": [], "on_wait": [w]},
            }
        )
    return nops


def _fix_module_json(raw: bytes) -> bytes:
    m = json.loads(raw)
    changed = False
    for f in m.get("functions", []):
        for b in f.get("blocks", []):
            out = []
            for inst in b.get("instructions", []):
                nops = _split_instruction_waits(inst)
                if nops:
                    changed = True
                    out.extend(nops)
                out.append(inst)
            b["instructions"] = out
    return json.dumps(m).encode() if changed else raw


_patch_installed = [False]


def _install_bir_patch():
    if _patch_installed[0]:
        return
    _patch_installed[0] = True
    import concourse.bass as _bass

    _orig = _bass.Bass.to_json_bytes

    def _patched(self):
        return _fix_module_json(_orig(self))

    _bass.Bass.to_json_bytes = _patched


# ---------------------------------------------------------------------------
# Problem constants (hardcoded per contest contract)
# ---------------------------------------------------------------------------
B, L, H, D = 4, 2048, 12, 64
CHUNK = 128
G = L // CHUNK  # 16
N_CORES = 8
PAIRS = [(n, h) for n in range(B) for h in range(H)]  # 48
PER_CORE = len(PAIRS) // N_CORES  # 6
NPP = PER_CORE // 2  # 3 pair-pairs per core

# ---------------------------------------------------------------------------
# Engine-assignment knobs (tuned against TimelineSim)
# ---------------------------------------------------------------------------
# kn evacuation engine per (s, half): "act" | "dve"
KN_ENG = {(pp, s, half): "dve" for pp in range(NPP) for s in (0, 1)
          for half in (0, 1)}
for _pp in range(NPP):
    KN_ENG[(_pp, 0, 0)] = "act"
KN_ENG[(2, 1, 0)] = "act"
KN_ENG[(2, 1, 1)] = "act"
KN_ENG[(2, 0, 1)] = "act"
KN_ENG[(1, 1, 1)] = "act"
# score mask mode per (pp, s, h): "b" = ACT evac + DVE bf16 mul,
#   "a" = ACT evac + Pool affine_select, "d" = DVE direct fp32 mul
MASK_MODE = {(pp, s, h): "a" for pp in range(NPP) for s in (0, 1)
             for h in (0, 1)}
for _s in (0, 1):
    for _h in (0, 1):
        MASK_MODE[(NPP - 1, _s, _h)] = "b"
PHASE_ORDER = [(0, 0), (1, 0), (0, 1), (1, 1)]
INS_BUFS = 2
WORK_BUFS = 3
OUTSB_BUFS = 2


def _build_bass():
    import concourse.bass as bass
    import concourse.tile as tile
    import concourse.mybir as mybir

    fp32 = mybir.dt.float32
    bf16 = mybir.dt.bfloat16
    AF = mybir.ActivationFunctionType
    ALU = mybir.AluOpType

    nc = bass.Bass()
    qt = nc.dram_tensor("qt", [NPP, 128, L], bf16, kind="ExternalInput")
    kt = nc.dram_tensor("kt", [NPP, 128, L], bf16, kind="ExternalInput")
    vt = nc.dram_tensor("vt", [NPP, 128, 2, G, D], bf16, kind="ExternalInput")
    mask = nc.dram_tensor("mask", [128, CHUNK], bf16, kind="ExternalInput")
    ident = nc.dram_tensor("ident", [128, D], bf16, kind="ExternalInput")
    on = nc.dram_tensor("on", [NPP, 128, 2, G, D], bf16, kind="ExternalOutput")

    with tile.TileContext(nc) as tc:
        with (
            tc.tile_pool(name="singles", bufs=1) as singles,
            tc.tile_pool(name="ins", bufs=INS_BUFS) as ins,
            tc.tile_pool(name="work", bufs=WORK_BUFS) as work,
            tc.tile_pool(name="ps_knt", bufs=1, space="PSUM") as ps_knt,
            tc.tile_pool(name="ps_kv", bufs=1, space="PSUM") as ps_kv,
            tc.tile_pool(name="ps_kz", bufs=1, space="PSUM") as ps_kz,
            tc.tile_pool(name="ps_sc", bufs=1, space="PSUM") as ps_sc,
            tc.tile_pool(name="ps_num", bufs=2, space="PSUM") as ps_num,
        ):
            st = {}

            def proA(pp, split_k=False):
                """Issue input DMAs for pair-pair pp (k first: exp(k) is the
                head of the longest dependency chain). k and q share one
                [128, 2, L] tile."""
                d = st.setdefault(pp, {})
                kq = ins.tile([128, 2, L], bf16, tag="kq", name=f"kq{pp}")
                vv = ins.tile([128, 2, G, D], bf16, tag="vv", name=f"vv{pp}")
                if split_k:
                    # halves let exp(k) start after half the transfer
                    nc.sync.dma_start(out=kq[:, 0, 0 : L // 2],
                                      in_=kt[pp, :, 0 : L // 2])
                    nc.sync.dma_start(out=kq[:, 0, L // 2 : L],
                                      in_=kt[pp, :, L // 2 : L])
                else:
                    nc.sync.dma_start(out=kq[:, 0], in_=kt[pp])
                # q on the sync queue too: a DGE setup on the ACT queue
                # would delay exp dispatch by ~0.7us
                nc.sync.dma_start(out=kq[:, 1], in_=qt[pp])
                nc.sync.dma_start(out=vv[:], in_=vt[pp])
                d["kq"], d["vv"] = kq, vv

            def singles_init():
                d = st.setdefault("S", {})
                identT = singles.tile([128, D], bf16)
                nc.sync.dma_start(out=identT[:], in_=ident[:])
                maskb = singles.tile([128, CHUNK], bf16)
                nc.sync.dma_start(out=maskb[:], in_=mask[:])
                onesb = singles.tile([128, 1], bf16)
                nc.vector.memset(onesb[:], 1.0)
                negone = singles.tile([128, 1], fp32)
                nc.gpsimd.memset(negone[:], -1.0)
                d["negone"] = negone
                # merged-scan segment mask over free dims (h=2, m=64, g=8):
                # 0 at (h, m, g=0) resets the running state per (h, m) row,
                # giving independent per-h chunk prefixes in one scan.
                seg = singles.tile([128, 2, D, 8], bf16)
                nc.vector.memset(seg[:], 1.0)
                nc.vector.memset(seg[:, :, :, 0:1], 0.0)
                seg16 = singles.tile([128, G], bf16)
                nc.vector.memset(seg16[:], 1.0)
                nc.vector.memset(seg16[:, 0:1], 0.0)
                d["maskb"], d["identT"], d["onesb"] = maskb, identT, onesb
                d["seg"], d["seg16"] = seg, seg16

            def phi_exp(pp, t, half=None):
                """ACT: e = exp(x) straight from the DMA'd input."""
                d = st[pp]
                ti = 0 if t == "k" else 1
                if "e" not in d:
                    d["e"] = work.tile([128, 2, L], bf16, tag="e",
                                       name=f"e{pp}")
                cs = slice(None) if half is None else \
                    slice(half[0] * (L // half[1]),
                          (half[0] + 1) * (L // half[1]))
                nc.scalar.activation(out=d["e"][:, ti, cs],
                                     in_=d["kq"][:, ti, cs], func=AF.Exp,
                                     bias=st["S"]["negone"][:])

            def phi_mam(pp, t, half=None):
                """DVE in place: e = min(e,1); x = x+1; x = max(x, e) = phi.
                After this kq[:, 0] = kf, kq[:, 1] = qf."""
                d = st[pp]
                ti = 0 if t == "k" else 1
                cs = slice(None) if half is None else \
                    slice(half[0] * (L // half[1]),
                          (half[0] + 1) * (L // half[1]))
                e, kq = d["e"], d["kq"]
                nc.vector.tensor_scalar_min(out=e[:, ti, cs],
                                            in0=e[:, ti, cs], scalar1=1.0)
                nc.vector.tensor_max(out=kq[:, ti, cs], in0=kq[:, ti, cs],
                                     in1=e[:, ti, cs])

            def proPE_T(pp, s, half, hb):
                """Transposes of phi(k) chunks for one h-half of partition
                half s. One knp tile [128, 2(hb), 8, D] per pp: the two hb
                slots ping-pong within a single PSUM bank (region-level
                dependency tracking), so transpose/evac/kv overlap."""
                d = st[pp]
                S = st["S"]
                kq = d["kq"]
                if "knp" not in d:
                    d["knp"] = ps_knt.tile([128, 2, 8, D], bf16, tag="knp",
                                           name=f"knp{pp}")
                knp = d["knp"]
                d[f"knph{s}_{half}"] = hb
                po = D * s
                for i, g in enumerate(range(half * 8, half * 8 + 8)):
                    nc.tensor.transpose(
                        knp[:, hb, i, :],
                        kq[po : po + D, 0, g * CHUNK : (g + 1) * CHUNK],
                        S["identT"][po : po + D, :],
                    )

            def kn_evac(pp, s, half):
                d = st[pp]
                knp = d["knp"]
                hb = d[f"knph{s}_{half}"]
                key = f"kn{s}"
                if key not in d:
                    d[key] = ins.tile([128, G, D], bf16, tag=f"kn{s}",
                                      name=f"kn{pp}_{s}")
                kn = d[key]
                gs = slice(half * 8, half * 8 + 8)
                if KN_ENG[(pp, s, half)] == "act":
                    nc.scalar.copy(out=kn[:, gs], in_=knp[:, hb])
                else:
                    nc.vector.tensor_copy(out=kn[:, gs], in_=knp[:, hb])

            def proPE_KV(pp, s, half):
                """kv products for one h-half of partition half s into
                kvp[h, m, g] slots + chunk k1 sums into kzp[:, 0:G]."""
                d = st[pp]
                S = st["S"]
                vv, kn = d["vv"], d[f"kn{s}"]
                if "kvp" not in d:
                    d["kvp"] = ps_kv.tile([128, 2, D, 8], fp32, tag="kv",
                                          name=f"kv{pp}")
                    d["kzp"] = ps_kz.tile([128, 48], fp32, tag="kz",
                                          name=f"kz{pp}")
                kvp, kzp = d["kvp"], d["kzp"]
                po = D * s
                for g in range(half * 8, half * 8 + 8):
                    h, gh = divmod(g, 8)
                    nc.tensor.matmul(
                        kvp[po : po + D, h, :, gh], kn[:, g, :],
                        vv[:, s, g, :],
                        start=(gh == 0), stop=(gh == 7),
                        skip_group_check=True)
                    nc.tensor.matmul(
                        kzp[po : po + D, g : g + 1], kn[:, g, :],
                        S["onesb"][:],
                        start=(g == 0), stop=False, skip_group_check=True)

            def scans(pp):
                """DVE: one segmented prefix scan over (h, m, g) — the seg
                reset at g=0 per (h, m) gives independent per-h chunk
                prefixes — plus the k1c scan. The h=0 total is NOT folded
                into h=1 here; phases add it via an extra PE matmul."""
                d = st[pp]
                S = st["S"]
                kvp, kzp = d["kvp"], d["kzp"]
                kvs = ins.tile([128, 2, D, 8], bf16, tag="kvs",
                               name=f"kvs{pp}")
                k1c = work.tile([128, G], bf16, tag="k1c", name=f"k1c{pp}")
                # k1c first: it unblocks the z-inter matmuls and is tiny
                nc.vector.tensor_tensor_scan(
                    out=k1c[:], data0=S["seg16"][:], data1=kzp[:, 0:G],
                    initial=0.0, op0=ALU.mult, op1=ALU.add)
                nc.vector.tensor_tensor_scan(
                    out=kvs[:].rearrange("p h m g -> p (h m g)"),
                    data0=S["seg"][:].rearrange("p h m g -> p (h m g)"),
                    data1=kvp[:].rearrange("p h m g -> p (h m g)"),
                    initial=0.0, op0=ALU.mult, op1=ALU.add)
                d["kvs"], d["k1c"] = kvs, k1c

            def phase_block(pp, pi):
                """One phase: 8 chunks of half s, h-group h."""
                s, h = PHASE_ORDER[pi]
                d = st[pp]
                S = st["S"]
                kq = d["kq"]
                vv, kvs, k1c, kzp = d["vv"], d["kvs"], d["k1c"], d["kzp"]
                if "outsb" not in d:
                    d["outsb"] = ins.tile([128, 2, G, D], bf16, tag="outsb",
                                          name=f"outsb{pp}", bufs=OUTSB_BUFS)
                outsb = d["outsb"]
                po = D * s
                zc = 16 + 16 * s
                hg = slice(8 * h, 8 * h + 8)
                nums = ps_num.tile([128, 8, D], fp32, tag="num",
                                   name=f"num{pp}_{s}_{h}")
                scp = ps_sc.tile([128, 8, CHUNK], fp32, tag="sc",
                                 name=f"sc{pp}_{s}_{h}")
                for i in range(8):
                    g = 8 * h + i
                    cs = slice(g * CHUNK, (g + 1) * CHUNK)
                    nc.tensor.matmul(
                        scp[:, i, :], kq[po : po + D, 0, cs],
                        kq[po : po + D, 1, cs],
                        start=True, stop=True, skip_group_check=True)
                for i in range(8):
                    g = 8 * h + i
                    if g == 0:
                        continue
                    cs = slice(g * CHUNK, (g + 1) * CHUNK)
                    hp, ghp = divmod(g - 1, 8)
                    nc.tensor.matmul(
                        nums[:, i, :], kq[po : po + D, 1, cs],
                        kvs[po : po + D, hp, :, ghp],
                        start=(i == 0 or g == 1), stop=False,
                        skip_group_check=True)
                    if hp == 1:
                        # per-h prefixes: add the h=0 total for h=1 chunks
                        nc.tensor.matmul(
                            nums[:, i, :], kq[po : po + D, 1, cs],
                            kvs[po : po + D, 0, :, 7 : 8].rearrange(
                                "p m g -> p (m g)"),
                            start=False, stop=False,
                            skip_group_check=True)
                    nc.tensor.matmul(
                        kzp[:, zc + g : zc + g + 1],
                        kq[po : po + D, 1, cs],
                        k1c[po : po + D, g - 1 : g],
                        start=False, stop=False,
                        skip_group_check=True)
                scb = work.tile([128, 8, CHUNK], bf16, tag="scb",
                                name=f"scb{pp}_{s}_{h}")
                mmode = MASK_MODE[(pp, s, h)]
                if mmode == "d":
                    mb = S["maskb"][:].unsqueeze(1).broadcast_to(
                        [128, 8, CHUNK])
                    nc.vector.tensor_mul(out=scb[:], in0=scp[:], in1=mb)
                else:
                    scf = work.tile([128, 8, CHUNK], bf16, tag="scf",
                                    name=f"scf{pp}_{s}_{h}")
                    nc.scalar.copy(out=scf[:], in_=scp[:])
                    if mmode == "a":
                        nc.gpsimd.affine_select(
                            out=scb[:], in_=scf[:],
                            pattern=[[0, 8], [1, CHUNK]],
                            compare_op=ALU.is_ge, fill=0.0, base=0,
                            channel_multiplier=-1)
                    else:
                        mb = S["maskb"][:].unsqueeze(1).broadcast_to(
                            [128, 8, CHUNK])
                        nc.vector.tensor_mul(out=scb[:], in0=scf[:], in1=mb)
                # z-intra first: the reciprocal chain unblocks after the
                # 8 tiny rowsum matmuls instead of after the big intras
                for i in range(8):
                    g = 8 * h + i
                    nc.tensor.matmul(
                        kzp[:, zc + g : zc + g + 1], scb[:, i, :],
                        S["onesb"][:],
                        start=False,
                        stop=(s == 1 and g == G - 1),
                        skip_group_check=True)
                for i in range(8):
                    g = 8 * h + i
                    nc.tensor.matmul(
                        nums[:, i, :], scb[:, i, :],
                        vv[:, s, g, :],
                        start=False, stop=(i == 7),
                        skip_group_check=True)
                r4 = work.tile([128, 8], fp32, tag="r4",
                               name=f"r4{pp}_{s}_{h}")
                nc.vector.reciprocal(
                    out=r4[:],
                    in_=kzp[:, zc + 8 * h : zc + 8 * h + 8])
                rb = r4[:].unsqueeze(2).broadcast_to([128, 8, D])
                nc.vector.tensor_mul(out=outsb[:, s, hg],
                                     in0=nums[:], in1=rb)
                nc.sync.dma_start(out=on[pp, :, s, hg],
                                  in_=outsb[:, s, hg])

            # ---------------- emission schedule ----------------
            # Startup: pp0's k-path races ahead at half-tile granularity so
            # the transpose/kv/scan chain starts as early as possible, with
            # exp(q)/phi(q) filling ACT/DVE gaps.
            def pro_tail(pp):
                """Transpose/evac/kv chain. The s=1 transposes depend only
                on phi columns (not the s=0 chain), so the rounds go
                (s0,h0),(s1,h0),(s0,h1),(s1,h1) with alternating ping-pong
                slots."""
                proPE_T(pp, 0, 0, 0)
                kn_evac(pp, 0, 0)
                proPE_T(pp, 1, 0, 1)
                kn_evac(pp, 1, 0)
                proPE_KV(pp, 0, 0)
                proPE_T(pp, 0, 1, 0)
                proPE_KV(pp, 1, 0)
                kn_evac(pp, 0, 1)
                proPE_T(pp, 1, 1, 1)
                kn_evac(pp, 1, 1)
                proPE_KV(pp, 0, 1)
                proPE_KV(pp, 1, 1)
                scans(pp)

            proA(0, split_k=True)
            singles_init()
            phi_exp(0, "k", (0, 2))
            phi_mam(0, "k", (0, 2))
            phi_exp(0, "k", (1, 2))
            proPE_T(0, 0, 0, 0)
            kn_evac(0, 0, 0)
            phi_mam(0, "k", (1, 2))
            proPE_T(0, 1, 0, 1)
            kn_evac(0, 1, 0)
            proPE_KV(0, 0, 0)
            phi_exp(0, "q")
            proPE_T(0, 0, 1, 0)
            proPE_KV(0, 1, 0)
            kn_evac(0, 0, 1)
            phi_mam(0, "q")
            proPE_T(0, 1, 1, 1)
            kn_evac(0, 1, 1)
            proPE_KV(0, 0, 1)
            proPE_KV(0, 1, 1)
            scans(0)

            for pp in range(NPP):
                nxt = pp + 1 if pp + 1 < NPP else None
                if nxt is not None:
                    proA(nxt)
                phase_block(pp, 0)
                phase_block(pp, 1)
                if nxt is not None:
                    phi_exp(nxt, "k")
                    phi_mam(nxt, "k")
                phase_block(pp, 2)
                if nxt is not None:
                    phi_exp(nxt, "q")
                phase_block(pp, 3)
                if nxt is not None:
                    phi_mam(nxt, "q")
                    pro_tail(nxt)

    return nc


_cached = {}


def _prep_inputs(q, k, v):
    import ml_dtypes

    bf = ml_dtypes.bfloat16
    # [s, c] : 1 if s<=c (bf16 exact 0/1)
    maskarr = np.ascontiguousarray(
        np.tril(np.ones((CHUNK, CHUNK), np.float32)).T
    ).astype(bf)
    identarr = np.concatenate([np.eye(D, dtype=np.float32)] * 2, axis=0).astype(bf)
    in_maps = []
    for c in range(N_CORES):
        sel = PAIRS[c * PER_CORE : (c + 1) * PER_CORE]
        qtl = np.empty((NPP, 128, L), bf)
        ktl = np.empty((NPP, 128, L), bf)
        vtl = np.empty((NPP, 128, 2, G, D), bf)
        for j in range(NPP):
            for s in (0, 1):
                n, h = sel[2 * j + s]
                # ship x+1: saves the +1 DVE pass on device (exp uses
                # the ACT bias to undo the shift)
                qtl[j, D * s : D * s + D] = (q[n, :, h, :].T + 1.0).astype(bf)
                ktl[j, D * s : D * s + D] = (k[n, :, h, :].T + 1.0).astype(bf)
                vtl[j, :, s] = (
                    v[n, :, h, :].reshape(G, CHUNK, D).transpose(1, 0, 2)
                ).astype(bf)
        in_maps.append(
            {"qt": qtl, "kt": ktl, "vt": vtl, "mask": maskarr, "ident": identarr}
        )
    return in_maps


def kernel(q: np.ndarray, k: np.ndarray, v: np.ndarray) -> np.ndarray:
    _install_bir_patch()
    from concourse.bass_utils import run_bass_kernel_spmd

    if "nc" not in _cached:
        _cached["nc"] = _build_bass()
    nc = _cached["nc"]

    in_maps = _prep_inputs(q, k, v)
    try:
        res = run_bass_kernel_spmd(nc, in_maps, core_ids=list(range(N_CORES)))
    except ModuleNotFoundError:
        # BASS_TRACE=1 with no axon NTFF hook in the container: retry untraced
        os.environ["BASS_NEVER_TRACE"] = "1"
        res = run_bass_kernel_spmd(nc, in_maps, core_ids=list(range(N_CORES)))
    _cached["last_result"] = res

    out = np.empty((B, L, H, D), np.float32)
    for c in range(N_CORES):
        sel = PAIRS[c * PER_CORE : (c + 1) * PER_CORE]
        o = res.results[c]["on"]  # [NPP, 128, 2, G, D] bf16
        for j in range(NPP):
            for s in (0, 1):
                n, h = sel[2 * j + s]
                # [c, G, D] -> [L, D]
                out[n, :, h, :] = (
                    o[j, :, s].astype(np.float32).transpose(1, 0, 2).reshape(L, D)
                )
    return out
